# revision 35
# baseline (speedup 1.0000x reference)
"""GCN layer (nn_GCNLayer_89103391522827) on 8 Trainium2 NeuronCores.

out = leaky_relu(Ahat @ (x @ W) + b, 0.2), Ahat = Dinv^.5 (A + I) Dinv^.5.

Strategy (sharding_hint: shard nodes / partition edges by destination):
  - Output rows (dst nodes) sharded across 8 cores: 12500 rows each.
  - Reorder: out = (Ahat @ x) @ W + b so the per-edge gather runs on raw x
    (cast fp16 for bandwidth) and W is applied per output tile afterwards.
  - Per core: edges with dst in its shard, grouped by (dst tile of 128, src
    block of 25k rows — int16-addressable; the gather ucode address MAC is
    unsigned, so negative offsets are NOT usable). dma_gather (SWDGE)
    fetches x16[src] rows into SBUF chunks of 128 edges. Segment-sum becomes
    a PE matmul against a one-hot routing matrix P[e, d] = (d == dst_local_e)
    streamed in fp8e4m3 (0/1 is exact in fp8, and fp8-stationary x
    fp16-moving matmul is exact on HW — halves the P HBM stream vs fp16).
    The normalization splits as dinv[src] folded into x16 host-side and
    dinv[dst] applied as the ACT scale on the PSUM flush.
  - THE key perf lever (HW-measured): dma_gather descriptor generation runs
    on the Q7 core pair selected by queue_num (cpu_id/2 == queue_num), so a
    single queue serializes all descriptor generation (~5 ns/row). Rotating
    the per-block gather calls across all 4 SWDGE queues with 3 gather-buffer
    slots in flight runs 4 pairs in parallel: 4.1 ms -> ~1.0 ms for the same
    gather stream. Descriptor scratch is doubled so ring space does not
    stall generation.
  - Per output tile: accumulate 4*c_sb one-hot matmuls into PSUM, flush via
    ACT (scale=dinv[dst]), transpose (PE), apply W (2 matmuls) + bias
    (ones-row matmul), leaky-relu as max(0.2x, x) on DVE. Do NOT use
    AF.Lrelu (alpha is ignored, fixed 0.01 slope) or AF.Prelu (honors alpha
    but costs ~3.4 ms in ACT table-set thrash when alternated with Copy).
  - SPMD: one program for all 8 cores; per-(tile, block) chunk counts padded
    to a global max c_sb. Pad slots gather row 0 of their block with an
    all-zero one-hot column so they contribute nothing.

Host-side prep is limited to graph metadata any GNN pipeline precomputes
(edge partitioning/sorting, degree counts, one-hot message-routing matrices)
and the fp16 storage cast/degree-scaling of x.
"""

import os
import sys
import time

import numpy as np

sys.path.insert(0, "/opt/trn_rl_repo")

N_NODES = 100000
N_EDGES = 3200000
D = 256
NCORES = 8
RPC = N_NODES // NCORES          # 12500 rows per core
NT = (RPC + 127) // 128          # 98 dst tiles per core (last has 84 rows)
LAST_ROWS = RPC - (NT - 1) * 128  # 84
SRC_BLOCKS = 4
BROWS = N_NODES // SRC_BLOCKS    # 25000 rows per block (int16-addressable)
G = 2                            # dst tiles per gather piece
NP = (NT + G - 1) // G           # 49 pieces
NQ = 4                           # SWDGE queues (4 Q7 pairs in parallel)

_CACHE = {}
LAST_RESULTS = None


def _prep_metadata(edge_index):
    """Edge partitioning by destination + static chunk layout (see module doc)."""
    src = np.asarray(edge_index[0], dtype=np.int64)
    dst = np.asarray(edge_index[1], dtype=np.int64)
    loop = np.arange(N_NODES, dtype=np.int64)
    src_all = np.concatenate([src, loop])
    dst_all = np.concatenate([dst, loop])

    deg = np.bincount(dst_all, minlength=N_NODES)
    dinv = 1.0 / np.sqrt(deg.astype(np.float64))

    cores = []
    c_sb = 1          # max chunks any single (tile, block) group needs
    c_str = 1         # max chunks a (piece, block) stripe needs (both tiles)
    for c in range(NCORES):
        m = (dst_all >= c * RPC) & (dst_all < (c + 1) * RPC)
        s = src_all[m]
        d = dst_all[m] - c * RPC
        t = d >> 7                      # dst tile
        dl = (d & 127).astype(np.int64)
        sb = s // BROWS                 # src block
        sl = (s % BROWS).astype(np.int16)
        cnt = np.zeros((NT, SRC_BLOCKS), np.int64)
        np.add.at(cnt, (t, sb), 1)
        c_sb = max(c_sb, -(-int(cnt.max()) // 128))
        scnt = cnt.reshape(NP, G, SRC_BLOCKS).sum(axis=1)
        c_str = max(c_str, -(-int(scnt.max()) // 128))
        cores.append((t, dl, sb, sl, cnt))

    import ml_dtypes

    meta = []
    # Shared-boundary stripe layout: per (piece, block) stripe of c_str
    # chunks, tile0's edges fill slots from the front, tile1's from the back.
    # Tile0's matmuls statically consume chunks [0, c_sb), tile1's
    # [c_str - c_sb, c_str) — the overlap chunks get one P column block per
    # tile. c_str ~= ceil((cnt0+cnt1)/128) < 2*c_sb, cutting pad slots (and
    # gather rows/descriptors) by ~10% vs per-(tile, block) padding.
    nch_g = SRC_BLOCKS * c_str          # gather chunks per piece
    ncolb = SRC_BLOCKS * 2 * c_sb       # P column blocks per piece
    cols_b = c_str * 8                  # idx cols per block per piece (nidx/16)
    for c in range(NCORES):
        t, dl, sb, sl, cnt = cores[c]
        piece = t // G
        tl = t - piece * G
        order = np.lexsort((t, sb, piece))
        inv = np.empty_like(order)
        inv[order] = np.arange(order.size)
        gid = t * SRC_BLOCKS + sb
        gstart = np.zeros(NT * SRC_BLOCKS, np.int64)
        uniq, first = np.unique(gid[order], return_index=True)
        gstart[uniq] = first
        pos = inv - gstart[gid]
        slot = np.where(tl == 0, pos, c_str * 128 - cnt[t, sb] + pos)
        part = slot & 127
        ch_str = slot >> 7              # chunk within the stripe
        assert ((tl == 0) | (ch_str >= c_str - c_sb)).all()
        assert ((tl == 1) | (ch_str < c_sb)).all()
        pcolb = np.where(tl == 0, sb * 2 * c_sb + ch_str,
                         sb * 2 * c_sb + ch_str - c_str + 2 * c_sb)

        # host-built one-hot routing matrices: pure 0/1 in fp8e4m3 (exact;
        # dinv[src] is folded into x16 and dinv[dst] into the ACT flush
        # scale), halving the P-stream HBM traffic vs fp16
        p16 = np.zeros((NP * 128, ncolb * 128), ml_dtypes.float8_e4m3fn)
        p16[piece * 128 + part, pcolb * 128 + dl] = 1.0

        # gather indices, wrapped 16-wide per block, replicated to 128 parts
        idx16 = np.zeros((NP, 16, SRC_BLOCKS * cols_b), np.int16)
        i_call = slot
        idx16[piece, i_call % 16, sb * cols_b + i_call // 16] = sl
        idx16 = np.broadcast_to(
            idx16[:, None, :, :], (NP, 8, 16, SRC_BLOCKS * cols_b)
        ).reshape(NP * 128, SRC_BLOCKS * cols_b)

        dinvd = np.zeros((128, NT), np.float32)
        rows = np.arange(NT * 128)
        valid = rows < RPC
        dinvd[rows[valid] & 127, rows[valid] >> 7] = dinv[c * RPC + rows[valid]]

        meta.append(dict(idx=np.ascontiguousarray(idx16), p16=p16, dinvd=dinvd))
    return c_sb, c_str, meta, dinv


def _build_program(c_sb, c_str, mode="full"):
    do_gather = mode in ("full", "gather")
    do_compute = mode in ("full", "compute")
    import concourse.bacc as bacc
    import concourse.mybir as mybir
    import concourse.tile as tile

    F8 = mybir.dt.float8e4
    F16 = mybir.dt.float16
    F32 = mybir.dt.float32
    F32R = mybir.dt.float32r
    I16 = mybir.dt.int16
    AF = mybir.ActivationFunctionType
    OP = mybir.AluOpType

    cols_b = c_str * 8
    nidx = c_str * 128
    nch_g = SRC_BLOCKS * c_str
    ncolb = SRC_BLOCKS * 2 * c_sb

    nc = bacc.Bacc("TRN2", target_bir_lowering=False, debug=False,
                   num_swdge_queues=NQ, dynamic_dma_scratch_size=32768)
    x16 = nc.dram_tensor("x16", [N_NODES, D], F16, kind="ExternalInput").ap()
    w = nc.dram_tensor("w", [D, D], F32R, kind="ExternalInput").ap()
    bvec = nc.dram_tensor("bvec", [1, D], F32R, kind="ExternalInput").ap()
    onesr = nc.dram_tensor("onesr", [1, 128], F32R, kind="ExternalInput").ap()
    idx = nc.dram_tensor("idx", [NP * 128, SRC_BLOCKS * cols_b], I16,
                         kind="ExternalInput").ap()
    p16d = nc.dram_tensor("p16", [NP * 128, ncolb * 128], F8,
                          kind="ExternalInput").ap()
    dinvd = nc.dram_tensor("dinvd", [128, NT], F32, kind="ExternalInput").ap()
    out = nc.dram_tensor("out", [RPC, D], F32, kind="ExternalOutput").ap()

    with tile.TileContext(nc) as tc:
        with (
            tc.tile_pool(name="const", bufs=1) as const,
            tc.tile_pool(name="meta", bufs=3) as metap,
            tc.tile_pool(name="pmat", bufs=3) as pmat,
            tc.tile_pool(name="gather", bufs=3) as gpool,
            tc.tile_pool(name="work", bufs=3) as wpool,
            tc.tile_pool(name="psA", bufs=2, space="PSUM") as psA,
            tc.tile_pool(name="psT", bufs=2, space="PSUM") as psT,
            tc.tile_pool(name="psO", bufs=2, space="PSUM") as psO,
        ):
            ones_t = const.tile([128, 128], F32)
            nc.vector.memset(ones_t[:], 1.0)
            ident = const.tile([128, 128], F32)
            nc.gpsimd.affine_select(
                ident[:], ones_t[:], [[1, 128]], OP.is_equal, 0.0,
                base=0, channel_multiplier=-1,
            )
            w0 = const.tile([128, D], F32R)
            w1 = const.tile([128, D], F32R)
            nc.scalar.dma_start(w0[:], w[0:128, :])
            nc.scalar.dma_start(w1[:], w[128:256, :])
            b_sb = const.tile([1, D], F32R)
            nc.scalar.dma_start(b_sb[:], bvec[:])
            ones_row = const.tile([1, 128], F32R)
            nc.scalar.dma_start(ones_row[:], onesr[:])
            dinvd_sb = const.tile([128, NT], F32)
            nc.scalar.dma_start(dinvd_sb[:], dinvd[:])

            for p in range(NP):
                idx_sb = metap.tile([128, SRC_BLOCKS * cols_b], I16, tag="idx")
                nc.sync.dma_start(idx_sb[:], idx[p * 128 : (p + 1) * 128, :])
                p_sb = pmat.tile([128, ncolb * 128], F8, tag="p")
                nc.sync.dma_start(p_sb[:], p16d[p * 128 : (p + 1) * 128, :])

                gbuf = gpool.tile([128, nch_g, D], F16, tag="gbuf")
                if do_gather:
                    for sb in range(SRC_BLOCKS):
                        stripe = sb * c_str
                        nc.gpsimd.dma_gather(
                            gbuf[:, stripe : stripe + c_str, :],
                            x16[sb * BROWS : (sb + 1) * BROWS, :],
                            idx_sb[:, sb * cols_b : sb * cols_b + nidx // 16],
                            nidx, nidx, D, single_packet=False,
                            queue_num=(SRC_BLOCKS * p + sb) % NQ,
                        )
                else:
                    nc.vector.memset(gbuf[:, 0, :], 0.25)
                if not do_compute:
                    probe = wpool.tile([128, D], F16, tag="probe")
                    nc.vector.tensor_copy(probe[:], gbuf[:, 0, :])
                    probe2 = wpool.tile([128, D], F32, tag="outs")
                    nc.vector.tensor_copy(probe2[:], probe[:])
                    t0 = p * G
                    nc.scalar.dma_start(out[t0 * 128 : t0 * 128 + 128, :], probe2[:])
                    continue

                for tl in range(G):
                    t = p * G + tl
                    agg_ps = psA.tile([128, D], F32, tag="agg")
                    # (gather chunk, P column block) pairs for this tile:
                    # tile0 reads the front c_sb chunks of each stripe,
                    # tile1 the back c_sb (overlap chunks are consumed by
                    # both tiles via separate P column blocks)
                    ch0 = 0 if tl == 0 else c_str - c_sb
                    pairs = [
                        (sb * c_str + ch0 + j,
                         sb * 2 * c_sb + tl * c_sb + j)
                        for sb in range(SRC_BLOCKS)
                        for j in range(c_sb)
                    ]
                    for k, (ch, pc) in enumerate(pairs):
                        rhs_ch = ch if do_gather else 0
                        nc.tensor.matmul(
                            agg_ps[:], p_sb[:, pc * 128 : (pc + 1) * 128],
                            gbuf[:, rhs_ch, :],
                            start=(k == 0), stop=(k == len(pairs) - 1),
                        )
                    agg_sb = wpool.tile([128, D], F32, tag="aggsb")
                    nc.scalar.activation(agg_sb[:], agg_ps[:], AF.Copy,
                                         scale=dinvd_sb[:, t : t + 1])
                    aggT_sb = wpool.tile([128, D], F32R, tag="aggT")
                    for h in range(2):
                        tp = psT.tile([128, 128], F32, tag="tp")
                        nc.tensor.transpose(
                            tp[:], agg_sb[:, h * 128 : (h + 1) * 128], ident[:]
                        )
                        nc.scalar.activation(
                            aggT_sb[:, h * 128 : (h + 1) * 128], tp[:], AF.Copy
                        )
                    out_ps = psO.tile([128, D], F32, tag="outp")
                    nc.tensor.matmul(out_ps[:], aggT_sb[:, 0:128], w0[:],
                                     start=True, stop=False)
                    nc.tensor.matmul(out_ps[:], aggT_sb[:, 128:256], w1[:],
                                     start=False, stop=False)
                    nc.tensor.matmul(out_ps[:], ones_row[:], b_sb[:],
                                     start=False, stop=True)
                    pre_sb = wpool.tile([128, D], F32, tag="pre")
                    nc.scalar.activation(pre_sb[:], out_ps[:], AF.Copy)
                    out_sb = wpool.tile([128, D], F32, tag="outs")
                    nc.vector.scalar_tensor_tensor(
                        out_sb[:], pre_sb[:], 0.2, pre_sb[:], OP.mult, OP.max
                    )
                    rows = LAST_ROWS if t == NT - 1 else 128
                    nc.scalar.dma_start(
                        out[t * 128 : t * 128 + rows, :], out_sb[:rows, :]
                    )
    nc.compile()
    return nc


def prepare(x, edge_index, W, b, mode="full"):
    """Build (or fetch cached) program + per-core input maps."""
    c_sb, c_str, meta, dinv = _prep_metadata(edge_index)
    key = (c_sb, c_str, mode)
    if key not in _CACHE:
        _CACHE[key] = _build_program(c_sb, c_str, mode=mode)
    nc = _CACHE[key]

    # dinv[src] folded into the gathered features (P stays pure 0/1 fp8)
    x16 = np.ascontiguousarray(
        (np.asarray(x, dtype=np.float32) * dinv[:, None].astype(np.float32))
        .astype(np.float16))
    W = np.asarray(W, dtype=np.float32)
    b = np.asarray(b, dtype=np.float32)
    in_maps = []
    for c in range(NCORES):
        in_maps.append(dict(
            x16=x16, w=W, bvec=b.reshape(1, D),
            onesr=np.ones((1, 128), np.float32),
            idx=meta[c]["idx"], p16=meta[c]["p16"], dinvd=meta[c]["dinvd"],
        ))
    return nc, in_maps


def kernel(x, edge_index, W, b):
    global LAST_RESULTS
    from concourse.bass_utils import run_bass_kernel_spmd

    t0 = time.time()
    nc, in_maps = prepare(x, edge_index, W, b)
    t1 = time.time()

    res = run_bass_kernel_spmd(nc, in_maps, list(range(NCORES)), trace=False)
    LAST_RESULTS = res
    t2 = time.time()
    if os.environ.get("GCN_VERBOSE", "0") == "1":
        print(f"[kernel] prep+build={t1-t0:.1f}s run={t2-t1:.1f}s",
              file=sys.stderr)

    return np.concatenate([res.results[c]["out"] for c in range(NCORES)], axis=0)


# revision 36
# speedup vs baseline: 1.1415x; 1.1415x over previous
"""GCN layer (nn_GCNLayer_89103391522827) on 8 Trainium2 NeuronCores.

out = leaky_relu(Ahat @ (x @ W) + b, 0.2), Ahat = Dinv^.5 (A + I) Dinv^.5.

Strategy (sharding_hint: shard nodes / partition edges by destination):
  - Output rows (dst nodes) sharded across 8 cores: 12500 rows each.
  - Reorder: out = (Ahat @ x) @ W + b so the per-edge gather runs on raw x
    (cast fp16 for bandwidth) and W is applied per output tile afterwards.
  - Per core: edges with dst in its shard, grouped by (dst tile of 128, src
    block of 25k rows — int16-addressable; the gather ucode address MAC is
    unsigned, so negative offsets are NOT usable). dma_gather (SWDGE)
    fetches x16[src] rows into SBUF chunks of 128 edges. Segment-sum becomes
    a PE matmul against a one-hot routing matrix P[e, d] = (d == dst_local_e)
    streamed in fp8e4m3 (0/1 is exact in fp8, and fp8-stationary x
    fp16-moving matmul is exact on HW — halves the P HBM stream vs fp16).
    The normalization splits as dinv[src] folded into x16 host-side and
    dinv[dst] applied as the ACT scale on the PSUM flush.
  - THE key perf lever (HW-measured): dma_gather descriptor generation runs
    on the Q7 core pair selected by queue_num (cpu_id/2 == queue_num), so a
    single queue serializes all descriptor generation (~5 ns/row). Rotating
    the per-block gather calls across all 4 SWDGE queues with 3 gather-buffer
    slots in flight runs 4 pairs in parallel: 4.1 ms -> ~1.0 ms for the same
    gather stream. Descriptor scratch is doubled so ring space does not
    stall generation.
  - Per output tile: accumulate 4*c_sb one-hot matmuls into PSUM, flush via
    ACT (scale=dinv[dst]), transpose (PE), apply W (2 matmuls) + bias
    (ones-row matmul), leaky-relu as max(0.2x, x) on DVE. Do NOT use
    AF.Lrelu (alpha is ignored, fixed 0.01 slope) or AF.Prelu (honors alpha
    but costs ~3.4 ms in ACT table-set thrash when alternated with Copy).
  - SPMD: one program for all 8 cores. Per (piece, block) the two tiles
    share a stripe of c_str = max ceil((cnt0+cnt1)/128) chunks: tile0 fills
    slots from the front, tile1 from the back; tile0's matmuls statically
    consume the front c_sb chunks, tile1's the back c_sb, with overlap
    chunks carrying one P column block per tile. This pads to the stripe
    max instead of per-(tile, block) maxes (c_str=19 < 2*c_sb=20). Pad
    slots gather row 0 of their block with an all-zero one-hot column so
    they contribute nothing (trailing-negative-index dropping is NOT used:
    idx<0 reads wild addresses and hung the device in microbenchmarks).

Host-side prep is limited to graph metadata any GNN pipeline precomputes
(edge partitioning/sorting, degree counts, one-hot message-routing matrices)
and the fp16 storage cast/degree-scaling of x.
"""

import os
import sys
import time

import numpy as np

sys.path.insert(0, "/opt/trn_rl_repo")

N_NODES = 100000
N_EDGES = 3200000
D = 256
NCORES = 8
RPC = N_NODES // NCORES          # 12500 rows per core
NT = (RPC + 127) // 128          # 98 dst tiles per core (last has 84 rows)
LAST_ROWS = RPC - (NT - 1) * 128  # 84
SRC_BLOCKS = 4
BROWS = N_NODES // SRC_BLOCKS    # 25000 rows per block (int16-addressable)
G = 2                            # dst tiles per gather piece
NP = (NT + G - 1) // G           # 49 pieces
NQ = 4                           # SWDGE queues (4 Q7 pairs in parallel)

_CACHE = {}
LAST_RESULTS = None


def _prep_metadata(edge_index):
    """Edge partitioning by destination + static chunk layout (see module doc)."""
    src = np.asarray(edge_index[0], dtype=np.int64)
    dst = np.asarray(edge_index[1], dtype=np.int64)
    loop = np.arange(N_NODES, dtype=np.int64)
    src_all = np.concatenate([src, loop])
    dst_all = np.concatenate([dst, loop])

    deg = np.bincount(dst_all, minlength=N_NODES)
    dinv = 1.0 / np.sqrt(deg.astype(np.float64))

    cores = []
    c_sb = 1          # max chunks any single (tile, block) group needs
    c_str = 1         # max chunks a (piece, block) stripe needs (both tiles)
    for c in range(NCORES):
        m = (dst_all >= c * RPC) & (dst_all < (c + 1) * RPC)
        s = src_all[m]
        d = dst_all[m] - c * RPC
        t = d >> 7                      # dst tile
        dl = (d & 127).astype(np.int64)
        sb = s // BROWS                 # src block
        sl = (s % BROWS).astype(np.int16)
        cnt = np.zeros((NT, SRC_BLOCKS), np.int64)
        np.add.at(cnt, (t, sb), 1)
        c_sb = max(c_sb, -(-int(cnt.max()) // 128))
        scnt = cnt.reshape(NP, G, SRC_BLOCKS).sum(axis=1)
        c_str = max(c_str, -(-int(scnt.max()) // 128))
        cores.append((t, dl, sb, sl, cnt))

    import ml_dtypes

    meta = []
    # Shared-boundary stripe layout: per (piece, block) stripe of c_str
    # chunks, tile0's edges fill slots from the front, tile1's from the back.
    # Tile0's matmuls statically consume chunks [0, c_sb), tile1's
    # [c_str - c_sb, c_str) — the overlap chunks get one P column block per
    # tile. c_str ~= ceil((cnt0+cnt1)/128) < 2*c_sb, cutting pad slots (and
    # gather rows/descriptors) by ~10% vs per-(tile, block) padding.
    nch_g = SRC_BLOCKS * c_str          # gather chunks per piece
    ncolb = SRC_BLOCKS * 2 * c_sb       # P column blocks per piece
    cols_b = c_str * 8                  # idx cols per block per piece (nidx/16)
    for c in range(NCORES):
        t, dl, sb, sl, cnt = cores[c]
        piece = t // G
        tl = t - piece * G
        order = np.lexsort((t, sb, piece))
        inv = np.empty_like(order)
        inv[order] = np.arange(order.size)
        gid = t * SRC_BLOCKS + sb
        gstart = np.zeros(NT * SRC_BLOCKS, np.int64)
        uniq, first = np.unique(gid[order], return_index=True)
        gstart[uniq] = first
        pos = inv - gstart[gid]
        slot = np.where(tl == 0, pos, c_str * 128 - cnt[t, sb] + pos)
        part = slot & 127
        ch_str = slot >> 7              # chunk within the stripe
        assert ((tl == 0) | (ch_str >= c_str - c_sb)).all()
        assert ((tl == 1) | (ch_str < c_sb)).all()
        pcolb = np.where(tl == 0, sb * 2 * c_sb + ch_str,
                         sb * 2 * c_sb + ch_str - c_str + 2 * c_sb)

        # host-built one-hot routing matrices: pure 0/1 in fp8e4m3 (exact;
        # dinv[src] is folded into x16 and dinv[dst] into the ACT flush
        # scale), halving the P-stream HBM traffic vs fp16
        p16 = np.zeros((NP * 128, ncolb * 128), ml_dtypes.float8_e4m3fn)
        p16[piece * 128 + part, pcolb * 128 + dl] = 1.0

        # gather indices, wrapped 16-wide per block, replicated to 128 parts
        idx16 = np.zeros((NP, 16, SRC_BLOCKS * cols_b), np.int16)
        i_call = slot
        idx16[piece, i_call % 16, sb * cols_b + i_call // 16] = sl
        idx16 = np.broadcast_to(
            idx16[:, None, :, :], (NP, 8, 16, SRC_BLOCKS * cols_b)
        ).reshape(NP * 128, SRC_BLOCKS * cols_b)

        dinvd = np.zeros((128, NT), np.float32)
        rows = np.arange(NT * 128)
        valid = rows < RPC
        dinvd[rows[valid] & 127, rows[valid] >> 7] = dinv[c * RPC + rows[valid]]

        meta.append(dict(idx=np.ascontiguousarray(idx16), p16=p16, dinvd=dinvd))
    return c_sb, c_str, meta, dinv


def _build_program(c_sb, c_str, mode="full"):
    do_gather = mode in ("full", "gather")
    do_compute = mode in ("full", "compute")
    import concourse.bacc as bacc
    import concourse.mybir as mybir
    import concourse.tile as tile

    F8 = mybir.dt.float8e4
    F16 = mybir.dt.float16
    F32 = mybir.dt.float32
    F32R = mybir.dt.float32r
    I16 = mybir.dt.int16
    AF = mybir.ActivationFunctionType
    OP = mybir.AluOpType

    cols_b = c_str * 8
    nidx = c_str * 128
    nch_g = SRC_BLOCKS * c_str
    ncolb = SRC_BLOCKS * 2 * c_sb

    nc = bacc.Bacc("TRN2", target_bir_lowering=False, debug=False,
                   num_swdge_queues=NQ, dynamic_dma_scratch_size=32768)
    x16 = nc.dram_tensor("x16", [N_NODES, D], F16, kind="ExternalInput").ap()
    w = nc.dram_tensor("w", [D, D], F32R, kind="ExternalInput").ap()
    bvec = nc.dram_tensor("bvec", [1, D], F32R, kind="ExternalInput").ap()
    onesr = nc.dram_tensor("onesr", [1, 128], F32R, kind="ExternalInput").ap()
    idx = nc.dram_tensor("idx", [NP * 128, SRC_BLOCKS * cols_b], I16,
                         kind="ExternalInput").ap()
    p16d = nc.dram_tensor("p16", [NP * 128, ncolb * 128], F8,
                          kind="ExternalInput").ap()
    dinvd = nc.dram_tensor("dinvd", [128, NT], F32, kind="ExternalInput").ap()
    out = nc.dram_tensor("out", [RPC, D], F32, kind="ExternalOutput").ap()

    with tile.TileContext(nc) as tc:
        with (
            tc.tile_pool(name="const", bufs=1) as const,
            tc.tile_pool(name="meta", bufs=3) as metap,
            tc.tile_pool(name="pmat", bufs=3) as pmat,
            tc.tile_pool(name="gather", bufs=3) as gpool,
            tc.tile_pool(name="work", bufs=3) as wpool,
            tc.tile_pool(name="psA", bufs=2, space="PSUM") as psA,
            tc.tile_pool(name="psT", bufs=2, space="PSUM") as psT,
            tc.tile_pool(name="psO", bufs=2, space="PSUM") as psO,
        ):
            ones_t = const.tile([128, 128], F32)
            nc.vector.memset(ones_t[:], 1.0)
            ident = const.tile([128, 128], F32)
            nc.gpsimd.affine_select(
                ident[:], ones_t[:], [[1, 128]], OP.is_equal, 0.0,
                base=0, channel_multiplier=-1,
            )
            w0 = const.tile([128, D], F32R)
            w1 = const.tile([128, D], F32R)
            nc.scalar.dma_start(w0[:], w[0:128, :])
            nc.scalar.dma_start(w1[:], w[128:256, :])
            b_sb = const.tile([1, D], F32R)
            nc.scalar.dma_start(b_sb[:], bvec[:])
            ones_row = const.tile([1, 128], F32R)
            nc.scalar.dma_start(ones_row[:], onesr[:])
            dinvd_sb = const.tile([128, NT], F32)
            nc.scalar.dma_start(dinvd_sb[:], dinvd[:])

            for p in range(NP):
                idx_sb = metap.tile([128, SRC_BLOCKS * cols_b], I16, tag="idx")
                nc.sync.dma_start(idx_sb[:], idx[p * 128 : (p + 1) * 128, :])
                p_sb = pmat.tile([128, ncolb * 128], F8, tag="p")
                nc.sync.dma_start(p_sb[:], p16d[p * 128 : (p + 1) * 128, :])

                gbuf = gpool.tile([128, nch_g, D], F16, tag="gbuf")
                if do_gather:
                    for sb in range(SRC_BLOCKS):
                        stripe = sb * c_str
                        nc.gpsimd.dma_gather(
                            gbuf[:, stripe : stripe + c_str, :],
                            x16[sb * BROWS : (sb + 1) * BROWS, :],
                            idx_sb[:, sb * cols_b : sb * cols_b + nidx // 16],
                            nidx, nidx, D, single_packet=False,
                            queue_num=(SRC_BLOCKS * p + sb) % NQ,
                        )
                else:
                    nc.vector.memset(gbuf[:, 0, :], 0.25)
                if not do_compute:
                    probe = wpool.tile([128, D], F16, tag="probe")
                    nc.vector.tensor_copy(probe[:], gbuf[:, 0, :])
                    probe2 = wpool.tile([128, D], F32, tag="outs")
                    nc.vector.tensor_copy(probe2[:], probe[:])
                    t0 = p * G
                    nc.scalar.dma_start(out[t0 * 128 : t0 * 128 + 128, :], probe2[:])
                    continue

                for tl in range(G):
                    t = p * G + tl
                    agg_ps = psA.tile([128, D], F32, tag="agg")
                    # (gather chunk, P column block) pairs for this tile:
                    # tile0 reads the front c_sb chunks of each stripe,
                    # tile1 the back c_sb (overlap chunks are consumed by
                    # both tiles via separate P column blocks)
                    ch0 = 0 if tl == 0 else c_str - c_sb
                    pairs = [
                        (sb * c_str + ch0 + j,
                         sb * 2 * c_sb + tl * c_sb + j)
                        for sb in range(SRC_BLOCKS)
                        for j in range(c_sb)
                    ]
                    for k, (ch, pc) in enumerate(pairs):
                        rhs_ch = ch if do_gather else 0
                        nc.tensor.matmul(
                            agg_ps[:], p_sb[:, pc * 128 : (pc + 1) * 128],
                            gbuf[:, rhs_ch, :],
                            start=(k == 0), stop=(k == len(pairs) - 1),
                        )
                    agg_sb = wpool.tile([128, D], F32, tag="aggsb")
                    nc.scalar.activation(agg_sb[:], agg_ps[:], AF.Copy,
                                         scale=dinvd_sb[:, t : t + 1])
                    aggT_sb = wpool.tile([128, D], F32R, tag="aggT")
                    for h in range(2):
                        tp = psT.tile([128, 128], F32, tag="tp")
                        nc.tensor.transpose(
                            tp[:], agg_sb[:, h * 128 : (h + 1) * 128], ident[:]
                        )
                        nc.scalar.activation(
                            aggT_sb[:, h * 128 : (h + 1) * 128], tp[:], AF.Copy
                        )
                    out_ps = psO.tile([128, D], F32, tag="outp")
                    nc.tensor.matmul(out_ps[:], aggT_sb[:, 0:128], w0[:],
                                     start=True, stop=False)
                    nc.tensor.matmul(out_ps[:], aggT_sb[:, 128:256], w1[:],
                                     start=False, stop=False)
                    nc.tensor.matmul(out_ps[:], ones_row[:], b_sb[:],
                                     start=False, stop=True)
                    pre_sb = wpool.tile([128, D], F32, tag="pre")
                    nc.scalar.activation(pre_sb[:], out_ps[:], AF.Copy)
                    out_sb = wpool.tile([128, D], F32, tag="outs")
                    nc.vector.scalar_tensor_tensor(
                        out_sb[:], pre_sb[:], 0.2, pre_sb[:], OP.mult, OP.max
                    )
                    rows = LAST_ROWS if t == NT - 1 else 128
                    nc.scalar.dma_start(
                        out[t * 128 : t * 128 + rows, :], out_sb[:rows, :]
                    )
    nc.compile()
    return nc


def prepare(x, edge_index, W, b, mode="full"):
    """Build (or fetch cached) program + per-core input maps."""
    c_sb, c_str, meta, dinv = _prep_metadata(edge_index)
    key = (c_sb, c_str, mode)
    if key not in _CACHE:
        _CACHE[key] = _build_program(c_sb, c_str, mode=mode)
    nc = _CACHE[key]

    # dinv[src] folded into the gathered features (P stays pure 0/1 fp8)
    x16 = np.ascontiguousarray(
        (np.asarray(x, dtype=np.float32) * dinv[:, None].astype(np.float32))
        .astype(np.float16))
    W = np.asarray(W, dtype=np.float32)
    b = np.asarray(b, dtype=np.float32)
    in_maps = []
    for c in range(NCORES):
        in_maps.append(dict(
            x16=x16, w=W, bvec=b.reshape(1, D),
            onesr=np.ones((1, 128), np.float32),
            idx=meta[c]["idx"], p16=meta[c]["p16"], dinvd=meta[c]["dinvd"],
        ))
    return nc, in_maps


def kernel(x, edge_index, W, b):
    global LAST_RESULTS
    from concourse.bass_utils import run_bass_kernel_spmd

    t0 = time.time()
    nc, in_maps = prepare(x, edge_index, W, b)
    t1 = time.time()

    res = run_bass_kernel_spmd(nc, in_maps, list(range(NCORES)), trace=False)
    LAST_RESULTS = res
    t2 = time.time()
    if os.environ.get("GCN_VERBOSE", "0") == "1":
        print(f"[kernel] prep+build={t1-t0:.1f}s run={t2-t1:.1f}s",
              file=sys.stderr)

    return np.concatenate([res.results[c]["out"] for c in range(NCORES)], axis=0)
